# revision 8
# baseline (speedup 1.0000x reference)
"""Cross-attention Trainium2 Bass kernel (v2 — transposed attn@V).

Sharding: B*T rows of `tokens` split across 8 cores (core c -> batch c//2,
t-rows [(c%2)*2048, +2048)).  Each core computes its full output slice;
host gather is pure concatenation.

Per-core pipeline (matmuls bf16, fp32 PSUM):
  inputs -> bf16 DRAM staging -> transpose DMA -> SBUF
  K^T = Wk^T ctx^T, V_aug = [ctx Wv | ones] per head, Q^T = Wq^T tok^T
  per head h, t-block of 1024:
    scores^T[s,t] = K_h^T.T Q_h^T   (PE, K=64)
    P = exp(scores/sqrt(d))         (ACT, bf16 out)
    ctx_aug[t, 65] = P^T.T V_aug    (PE, N=65; col 64 = softmax denom)
    nrm[t, d] = ctx * recip(denom)  (DVE per-partition scalar)
    ctxn[e, t] = PE-transpose(nrm)  (pairs of heads -> 128 e rows)
  out = ctxn^T Wo + bo  (PE + DVE bias add), interleaved with next block's
  heads so the ACT exp stream never starves.
"""

import math
from contextlib import ExitStack

import numpy as np

import concourse.bass as bass
import concourse.mybir as mybir
import concourse.tile as tile
from concourse import bacc, masks
from concourse.bass_utils import run_bass_kernel_spmd

B, T, S = 4, 4096, 1024
HID, CTX, EMB, H = 1024, 768, 1024, 16
D = EMB // H  # 64
NCORES = 8
TC = (B * T) // NCORES  # 2048 rows of tokens per core
SCALE = 1.0 / math.sqrt(D)

F32 = mybir.dt.float32
BF16 = mybir.dt.bfloat16
EXP = mybir.ActivationFunctionType.Exp


def build(debug=False, repeat=1):
    nc = bacc.Bacc("TRN2", target_bir_lowering=False, debug=False,
                   num_devices=NCORES)
    tokens = nc.dram_tensor("tokens", [TC, HID], F32, kind="ExternalInput")
    ctx_in = nc.dram_tensor("context", [S, CTX], F32, kind="ExternalInput")
    wq = nc.dram_tensor("Wq", [HID, EMB], F32, kind="ExternalInput")
    wk = nc.dram_tensor("Wk", [CTX, EMB], F32, kind="ExternalInput")
    wv = nc.dram_tensor("Wv", [CTX, EMB], F32, kind="ExternalInput")
    wo = nc.dram_tensor("Wo", [EMB, HID], F32, kind="ExternalInput")
    bo = nc.dram_tensor("bo", [HID], F32, kind="ExternalInput")
    out = nc.dram_tensor("out", [TC, HID], F32, kind="ExternalOutput")
    dbg = {}
    if debug:
        dbg["ctxn"] = nc.dram_tensor("dbg_ctxn", [2, EMB, 1024], F32,
                                     kind="ExternalOutput")

    with tile.TileContext(nc) as tc, ExitStack() as outer:
        wpool = outer.enter_context(tc.tile_pool(name="weights", bufs=1))
        qkv = outer.enter_context(tc.tile_pool(name="qkv", bufs=1))
        psum = outer.enter_context(tc.tile_pool(name="psum", bufs=1, space="PSUM"))

        ident = wpool.tile([128, 128], BF16, name="ident", tag="ident")
        masks.make_identity(nc, ident[:, :])

        def psum_big():
            return psum.tile([128, 1024], F32, name="big", tag="big", bufs=2)

        for _it in range(repeat):
            wk_sb = [wpool.tile([128, EMB], BF16, name=f"wk{i}", tag=f"wk{i}") for i in range(6)]
            wv_sb = [wpool.tile([128, EMB], BF16, name=f"wv{i}", tag=f"wv{i}") for i in range(6)]
            wq_sb = [wpool.tile([128, EMB], BF16, name=f"wq{i}", tag=f"wq{i}") for i in range(8)]
            kt = [qkv.tile([128, S], BF16, name=f"kt{i}", tag=f"kt{i}") for i in range(8)]
            v_sb = [qkv.tile([128, H, D + 1], BF16, name=f"v{i}", tag=f"v{i}") for i in range(8)]
            qt = [qkv.tile([128, TC], BF16, name=f"qt{i}", tag=f"qt{i}") for i in range(8)]

            with ExitStack() as proj:
                dram = proj.enter_context(tc.tile_pool(name="dram", bufs=1, space="DRAM"))
                tmp = proj.enter_context(tc.tile_pool(name="tmp", bufs=1))

                # Input preprocessing: fp32 load -> ACT cast -> bf16 DRAM
                # store -> HWDGE transpose load. (ACT is idle this phase.)
                fstage = proj.enter_context(tc.tile_pool(name="fstage", bufs=3))
                gstage = proj.enter_context(tc.tile_pool(name="gstage", bufs=3))
                ctx16 = dram.tile([S, CTX], BF16, name="ctx16", tag="ctx16")
                tok16 = dram.tile([TC, HID], BF16, name="tok16", tag="tok16")
                ctxT = [tmp.tile([128, S], BF16, name=f"ctxT{i}", tag=f"ctxT{i}") for i in range(6)]
                tokT = [tmp.tile([128, TC], BF16, name=f"tokT{i}", tag=f"tokT{i}") for i in range(8)]
                for r in range(8):
                    f = fstage.tile([128, CTX], F32, name="cf", tag="cf")
                    g = gstage.tile([128, CTX], BF16, name="cg", tag="cg")
                    nc.sync.dma_start(out=f[:, :], in_=ctx_in.ap()[r * 128:(r + 1) * 128, :])
                    nc.scalar.copy(g[:, :], f[:, :])
                    nc.scalar.dma_start(out=ctx16[r * 128:(r + 1) * 128, :], in_=g[:, :])
                for i in range(6):
                    nc.sync.dma_start(out=ctxT[i][:, :],
                                      in_=ctx16[:, i * 128:(i + 1) * 128], transpose=True)
                    nc.gpsimd.dma_start(out=wk_sb[i][:, :], in_=wk.ap()[i * 128:(i + 1) * 128, :])
                    nc.gpsimd.dma_start(out=wv_sb[i][:, :], in_=wv.ap()[i * 128:(i + 1) * 128, :])
                for r in range(16):
                    f = fstage.tile([128, HID], F32, name="tf", tag="tf")
                    g = gstage.tile([128, HID], BF16, name="tg", tag="tg")
                    nc.sync.dma_start(out=f[:, :], in_=tokens.ap()[r * 128:(r + 1) * 128, :])
                    nc.scalar.copy(g[:, :], f[:, :])
                    nc.scalar.dma_start(out=tok16[r * 128:(r + 1) * 128, :], in_=g[:, :])
                for i in range(8):
                    nc.sync.dma_start(out=tokT[i][:, :],
                                      in_=tok16[:, i * 128:(i + 1) * 128], transpose=True)
                    nc.gpsimd.dma_start(out=wq_sb[i][:, :], in_=wq.ap()[i * 128:(i + 1) * 128, :])

                # K^T[e, s]
                for e in range(8):
                    ps = psum_big()
                    for sh in range(2):
                        for c in range(6):
                            nc.tensor.matmul(ps[:, sh * 512:(sh + 1) * 512],
                                             lhsT=wk_sb[c][:, e * 128:(e + 1) * 128],
                                             rhs=ctxT[c][:, sh * 512:(sh + 1) * 512],
                                             start=(c == 0), stop=(c == 5))
                    nc.vector.tensor_copy(kt[e][:, :], ps[:, :])
                # V[s, d] (+ ones column per head)
                for sc in range(8):
                    ps = psum_big()
                    for dh in range(2):
                        for c in range(6):
                            nc.tensor.matmul(ps[:, dh * 512:(dh + 1) * 512],
                                             lhsT=ctxT[c][:, sc * 128:(sc + 1) * 128],
                                             rhs=wv_sb[c][:, dh * 512:(dh + 1) * 512],
                                             start=(c == 0), stop=(c == 5))
                    nc.vector.tensor_copy(
                        v_sb[sc][:, :, 0:D],
                        ps[:, :].rearrange("p (h d) -> p h d", d=D))
                    nc.vector.memset(v_sb[sc][:, :, D:D + 1], 1.0)
                # Q^T[e, t]
                for e in range(8):
                    for tq in range(2):
                        ps = psum_big()
                        for hc in range(8):
                            for sh in range(2):
                                nc.tensor.matmul(
                                    ps[:, sh * 512:(sh + 1) * 512],
                                    lhsT=wq_sb[hc][:, e * 128:(e + 1) * 128],
                                    rhs=tokT[hc][:, tq * 1024 + sh * 512:tq * 1024 + (sh + 1) * 512],
                                    start=(hc == 0), stop=(hc == 7))
                        nc.vector.tensor_copy(qt[e][:, tq * 1024:(tq + 1) * 1024], ps[:, :])

            # load Wo/bo while attention runs
            wo_sb = [wpool.tile([128, HID], BF16, name=f"wo{i}", tag=f"wo{i}") for i in range(8)]
            bo_b = wpool.tile([128, HID], F32, name="bo", tag="bo")
            for i in range(8):
                nc.gpsimd.dma_start(out=wo_sb[i][:, :], in_=wo.ap()[i * 128:(i + 1) * 128, :])
            b_ap = bo.ap()
            nc.gpsimd.dma_start(
                out=bo_b[:, :],
                in_=bass.AP(tensor=b_ap.tensor, offset=b_ap.offset,
                            ap=[[0, 128]] + list(b_ap.ap)))

            # ---------------- attention (head-pipelined) + output projection
            with ExitStack() as attn:
                ctxn_pool = attn.enter_context(tc.tile_pool(name="ctxn", bufs=2))
                apool = attn.enter_context(tc.tile_pool(name="apool", bufs=2))
                npool = attn.enter_context(tc.tile_pool(name="npool", bufs=2))
                opool = attn.enter_context(tc.tile_pool(name="opool", bufs=3))

                ctxn = {}       # (tb, p) -> [128 e, 1024 t] bf16
                nrm_tiles = {}  # (tb, p) -> [128 t, 8 tc, 128 e] bf16

                def scores_chunk(h, tb, sc):
                    """PE: scores^T[s-chunk, 1024 t]; ACT: exp -> pa bf16."""
                    p, hr = h // 2, (h % 2) * 64
                    t0 = tb * 1024
                    ps = psum_big()
                    for tt in range(2):
                        nc.tensor.matmul(
                            ps[:, tt * 512:(tt + 1) * 512],
                            lhsT=kt[p][hr:hr + 64, sc * 128:(sc + 1) * 128],
                            rhs=qt[p][hr:hr + 64, t0 + tt * 512:t0 + (tt + 1) * 512],
                            start=True, stop=True)
                    pt = apool.tile([128, 1024], BF16, name=f"pa{sc}", tag=f"pa{sc}")
                    nc.scalar.activation(pt[:, :], ps[:, :], EXP, scale=SCALE)
                    return pt

                def attnv_tc(st, tcb):
                    """PE: ctx_aug[t,65] for one t-chunk (own psum bank group);
                    DVE: divide by denominator col into nrm tile."""
                    h, tb, pa = st
                    p, hcol = h // 2, (h % 2) * 64
                    if (tb, p) not in nrm_tiles:
                        nrm_tiles[(tb, p)] = npool.tile(
                            [128, 8, 128], BF16, name=f"nrm{p}", tag="nrm", bufs=2)
                    nt = nrm_tiles[(tb, p)]
                    cps = psum.tile([128, D + 1], F32, name="ctx", tag="ctx",
                                    bufs=3)
                    for sc in range(8):
                        nc.tensor.matmul(
                            cps[:, :],
                            lhsT=pa[sc][:, tcb * 128:(tcb + 1) * 128],
                            rhs=v_sb[sc][:, h, :],
                            start=(sc == 0), stop=(sc == 7))
                    rc = npool.tile([128, 1], F32, name="rc", tag="rc", bufs=6)
                    nc.vector.reciprocal(rc[:, :], cps[:, D:D + 1])
                    nc.vector.tensor_scalar_mul(
                        nt[:, tcb, hcol:hcol + 64], cps[:, 0:D], rc[:, :])

                def transpose_pair(p, tb):
                    """PE transpose [t,128e]->[128e,t] per tc; DVE evac."""
                    nt = nrm_tiles[(tb, p)]
                    tp = psum.tile([128, 1024], BF16, name="tp", tag="tp", bufs=1)
                    for tcb in range(8):
                        nc.tensor.transpose(tp[:, tcb * 128:(tcb + 1) * 128],
                                            nt[:, tcb, :], ident[:, :])
                    cx = ctxn_pool.tile([128, 1024], BF16, name=f"cx{p}", tag=f"cx{p}")
                    nc.vector.tensor_copy(cx[:, :], tp[:, :])
                    ctxn[(tb, p)] = cx

                def outproj_chunk(tb, tck):
                    t0 = tb * 1024
                    ps = psum_big()
                    for nh in range(2):
                        ot = opool.tile([128, 512], F32, name="o", tag="o")
                        for p in range(8):
                            nc.tensor.matmul(ps[:, nh * 512:(nh + 1) * 512],
                                             lhsT=ctxn[(tb, p)][:, tck * 128:(tck + 1) * 128],
                                             rhs=wo_sb[p][:, nh * 512:(nh + 1) * 512],
                                             start=(p == 0), stop=(p == 7))
                        nc.vector.tensor_add(ot[:, :], ps[:, nh * 512:(nh + 1) * 512],
                                             bo_b[:, nh * 512:(nh + 1) * 512])
                        nc.sync.dma_start(
                            out=out.ap()[t0 + tck * 128:t0 + (tck + 1) * 128,
                                         nh * 512:(nh + 1) * 512],
                            in_=ot[:, :])

                pend = None  # (h, tb, pa-tiles) awaiting attnV
                for tb in range(2):
                    for h in range(16):
                        pa = []
                        for j in range(8):
                            pa.append(scores_chunk(h, tb, j))
                            if pend is not None:
                                attnv_tc(pend, j)
                        if pend is not None and pend[0] % 2 == 1:
                            transpose_pair(pend[0] // 2, pend[1])
                        pend = (h, tb, pa)
                        # interleave prev block's output projection
                        if tb == 1 and h < 8:
                            outproj_chunk(0, h)
                for j in range(8):
                    attnv_tc(pend, j)
                transpose_pair(7, 1)
                if debug:
                    for tbb in range(2):
                        for p in range(8):
                            nc.gpsimd.dma_start(
                                out=dbg["ctxn"].ap()[tbb, p * 128:(p + 1) * 128, :],
                                in_=ctxn[(tbb, p)][:, :])
                for tck in range(8):
                    outproj_chunk(1, tck)

    nc.compile()
    return nc


_CACHE = {}


def _get_nc(debug=False, repeat=1):
    key = (debug, repeat)
    if key not in _CACHE:
        _CACHE[key] = build(debug, repeat)
    return _CACHE[key]


def make_in_maps(tokens, context, Wq, Wk, Wv, Wo, bo):
    tokens = np.ascontiguousarray(np.asarray(tokens, dtype=np.float32))
    context = np.ascontiguousarray(np.asarray(context, dtype=np.float32))
    shared = {
        "Wq": np.ascontiguousarray(np.asarray(Wq, dtype=np.float32)),
        "Wk": np.ascontiguousarray(np.asarray(Wk, dtype=np.float32)),
        "Wv": np.ascontiguousarray(np.asarray(Wv, dtype=np.float32)),
        "Wo": np.ascontiguousarray(np.asarray(Wo, dtype=np.float32)),
        "bo": np.ascontiguousarray(np.asarray(bo, dtype=np.float32)),
    }
    in_maps = []
    for c in range(NCORES):
        b, t0 = c // 2, (c % 2) * TC
        in_maps.append({
            "tokens": np.ascontiguousarray(tokens[b, t0:t0 + TC, :]),
            "context": np.ascontiguousarray(context[b]),
            **shared,
        })
    return in_maps


def kernel(tokens, context, Wq, Wk, Wv, Wo, bo, _debug=False):
    nc = _get_nc(_debug)
    in_maps = make_in_maps(tokens, context, Wq, Wk, Wv, Wo, bo)
    res = run_bass_kernel_spmd(nc, in_maps, core_ids=list(range(NCORES)))
    out = np.empty((B, T, HID), dtype=np.float32)
    for c in range(NCORES):
        b, t0 = c // 2, (c % 2) * TC
        out[b, t0:t0 + TC, :] = res.results[c]["out"]
    if _debug:
        return out, res
    return out
